# revision 10
# baseline (speedup 1.0000x reference)
"""Trainium2 Bass kernel for nn_Clustering (discriminative/lane clustering loss).

Strategy (8 NeuronCores, data parallel over batch, 2 images per core):
  Per image b the loss needs only 24 per-cluster statistics (c = 1..4):
    counts_c = sum_px [inst==c]                      (4)
    S_ce     = sum_px [inst==c] * binary * pred_e    (16)
    T_c      = sum_px [inst==c] * binary * |pred|^2  (4)

  Gram formulation: the 20 masked products S/T are inner products between
  mask planes q_c = [inst==c]*binary and value planes {pred_e, r=|pred|^2}
  over all pixels.  Feed the masks as the PE *stationary* (4 masks x 32
  w-offsets = 128 columns, reloaded per 32-column block) and stream the
  values as *moving* data [5 channels x 32 offsets = 160 columns]; the
  (wa==wb) diagonal of the accumulated [128,160] PSUM Gram holds the
  statistics.  The 16 mask*pred multiplies happen inside the systolic
  array, cutting DVE work ~3x vs elementwise product planes.

  Engine split per [128, 1024] tile:
    DVE : int->bf16 cast, v = inst*binary, 8 indicator compares (4 masked
          q_c on v, 4 raw ind_c on inst), 2 adds for r = sum_e pred_e^2
    ACT : binary + pred f32->bf16 casts, pred^2 squares, PSUM evacuation
    PE  : 32 Gram matmuls [128,(4,32)]^T @ [128,(5,32)] per tile, plus
          ones-column count reductions in 4 concurrent column groups
    DMA : 3 HWDGE loads per tile, 1 store per image
  The host reduces the Gram diagonal and evaluates the tiny [B,C,E] tail
  (means, variance hinge, pairwise center repulsion).
"""
import sys

sys.path.insert(0, '/opt/trn_rl_repo')

import numpy as np
from contextlib import ExitStack

import concourse.bass as bass
import concourse.mybir as mybir
import concourse.tile as tile
from concourse.alu_op_type import AluOpType
from concourse.vector_clock import ScopedClock

F32 = mybir.dt.float32
I32 = mybir.dt.int32
BF16 = mybir.dt.bfloat16

B, E, H, W = 16, 4, 512, 1024
NCORES = 8
B_LOC = B // NCORES          # images per core
C = 4                        # clusters 1..4 (background dropped)
HT = H // 128                # h-tiles per image
WB = 32                      # gram block width (4 masks x 32 = 128 stationary)
NB = W // WB                 # gram blocks per tile row
NV = 5                       # moving channels: pred_e x4, r
NMM = 512                    # counts matmul moving width (one PSUM bank)
GW = NV * WB                 # gram psum cols = 160
OUTW = GW + NMM              # out cols: gram 160 + counts 512

DELTA_V = 0.5
DELTA_D = 3.0

# ---------------------------------------------------------------------------
# Toolchain workaround: this walrus build rejects instructions carrying more
# than one sem-wait ("Too many sync wait commands").  Keep 1 wait per
# instruction and spill the rest onto preceding same-engine NOPs (the engine
# executes them in order, so semantics are unchanged).
_MAX_WAITS = 1


def _split_waits_prepend(tc, inst):
    si = getattr(inst, 'sync_info', None)
    if si is None or not si.on_wait or len(si.on_wait) <= _MAX_WAITS:
        return
    if inst.engine == mybir.EngineType.Unassigned:
        return
    waits = list(si.on_wait)
    si.on_wait = waits[:_MAX_WAITS]
    inst.sync_info = si
    for i in range(_MAX_WAITS, len(waits), _MAX_WAITS):
        nop = mybir.InstNoOp(name=tc.nc.get_next_instruction_name(),
                             text_hint="wait_split")
        nop.engine = inst.engine
        nop.sync_info = mybir.SyncInfo(on_wait=waits[i:i + _MAX_WAITS],
                                       on_update=[])
        tc._add_instruction(nop)


_orig_commit_and_lower = tile.TileContext._commit_and_lower


def _patched_commit_and_lower(self, inst, original_block, old_bb_map,
                              bb_to_exit_bb):
    _split_waits_prepend(self, inst)
    return _orig_commit_and_lower(self, inst, original_block, old_bb_map,
                                  bb_to_exit_bb)


tile.TileContext._commit_and_lower = _patched_commit_and_lower


def _patched_drain_and_barrier(self, tick_clock, wait_clock):
    nc = self.nc
    drain_inst = nc.sync.drain()
    wait_clock.add_sem_waits(
        drain_inst.ins, ScopedClock({None: tick_clock.global_clock})
    )
    si = drain_inst.ins.sync_info
    if si is not None and si.on_wait and len(si.on_wait) > _MAX_WAITS:
        waits = list(si.on_wait)
        si.on_wait = waits[:_MAX_WAITS]
        drain_inst.ins.sync_info = si
        extra = waits[_MAX_WAITS:]
        for i in range(0, len(extra), _MAX_WAITS):
            nop = nc.sync.nop()
            nop.ins.sync_info = mybir.SyncInfo(
                on_wait=extra[i:i + _MAX_WAITS], on_update=[]
            )
    nc.all_engine_barrier()
    assert self.sems is not None
    popped = nc._tile_sem_poison_stack.pop()
    assert popped is self._sem_poison
    nc.clear_and_free_semaphores(list(self.sems.allocated().values()))
    nc.all_engine_barrier()


tile.TileContext._drain_and_barrier = _patched_drain_and_barrier
# ---------------------------------------------------------------------------


def _build_nc():
    nc = bass.Bass()
    pred = nc.dram_tensor("pred", [B_LOC, E, H, W], F32, kind="ExternalInput")
    binary = nc.dram_tensor("binary", [B_LOC, H, W], F32, kind="ExternalInput")
    inst = nc.dram_tensor("inst", [B_LOC, H, W], I32, kind="ExternalInput")
    out = nc.dram_tensor("out", [B_LOC, 128, OUTW], F32, kind="ExternalOutput")

    with tile.TileContext(nc) as tc:
        with ExitStack() as ctx:
            const_pool = ctx.enter_context(tc.tile_pool(name="const", bufs=1))
            in_pool = ctx.enter_context(tc.tile_pool(name="inp", bufs=3))
            bf_pool = ctx.enter_context(tc.tile_pool(name="bf", bufs=2))
            vals_pool = ctx.enter_context(tc.tile_pool(name="vals", bufs=2))
            mask_pool = ctx.enter_context(tc.tile_pool(name="mask", bufs=2))
            sq_pool = ctx.enter_context(tc.tile_pool(name="sq", bufs=2))
            ps_pool = ctx.enter_context(
                tc.tile_pool(name="ps", bufs=2, space="PSUM"))
            out_pool = ctx.enter_context(tc.tile_pool(name="outp", bufs=2))

            ones = const_pool.tile([128, 1], BF16)
            nc.vector.memset(ones[:], 1.0)

            for b in range(B_LOC):
                gram_ps = ps_pool.tile([128, GW], F32, tag="gram")
                counts_ps = ps_pool.tile([128, NMM], F32, tag="cnt")
                for t in range(HT):
                    h0 = 128 * t
                    inst_t = in_pool.tile([128, W], I32, tag="inst")
                    nc.sync.dma_start(
                        out=inst_t[:], in_=inst[b, h0:h0 + 128, :])
                    # f32 -> bf16 casts happen in-flight on the SWDGE DMA
                    # path, keeping ACT off the critical path.
                    bin_bf = bf_pool.tile([128, W], BF16, tag="binbf")
                    nc.gpsimd.dma_start(
                        out=bin_bf[:], in_=binary[b, h0:h0 + 128, :])
                    vals = vals_pool.tile([128, NV, W], BF16, tag="vals")
                    nc.gpsimd.dma_start(
                        out=vals[:, 0:E],
                        in_=pred[b, :, h0:h0 + 128, :].rearrange(
                            "e h w -> h e w"),
                    )

                    # ACT: squares + int cast
                    sq = sq_pool.tile([128, E, W], BF16, tag="sq")
                    nc.scalar.activation(
                        sq[:], vals[:, 0:E],
                        mybir.ActivationFunctionType.Square)
                    inst_bf = bf_pool.tile([128, W], BF16, tag="instbf")
                    nc.scalar.copy(inst_bf[:], inst_t[:])
                    v = bf_pool.tile([128, W], BF16, tag="v")
                    nc.vector.tensor_tensor(v[:], inst_bf[:], bin_bf[:],
                                            AluOpType.mult)
                    masks_q = mask_pool.tile([128, NB, C, WB], BF16, tag="mq")
                    vq = v[:].rearrange("p (blk w) -> p blk w", w=WB)
                    for c in range(C):
                        nc.vector.tensor_scalar(
                            masks_q[:, :, c, :], vq, float(c + 1), None,
                            AluOpType.is_equal)
                    ind = mask_pool.tile([128, C, W], BF16, tag="ind")
                    for c in range(C):
                        nc.vector.tensor_scalar(
                            ind[:, c], inst_bf[:], float(c + 1), None,
                            AluOpType.is_equal)
                    r2 = sq_pool.tile([128, 2, W], BF16, tag="r2")
                    nc.vector.tensor_tensor(r2[:], sq[:, 0:2], sq[:, 2:4],
                                            AluOpType.add)
                    nc.vector.tensor_tensor(vals[:, E], r2[:, 0], r2[:, 1],
                                            AluOpType.add)

                    # PE: Gram blocks — masks^T @ vals, diagonal-extracted on
                    # the host.  Accumulates over h-tiles and w-blocks.
                    for wb in range(NB):
                        w0 = WB * wb
                        nc.tensor.matmul(
                            gram_ps[:],
                            masks_q[:, wb],
                            vals[:, :, w0:w0 + WB],
                            start=(t == 0 and wb == 0),
                            stop=(t == HT - 1 and wb == NB - 1),
                        )
                    # PE: counts — ones-column partition reductions of the
                    # 4 raw indicator planes, 4 concurrent column groups.
                    for ch in range(W // NMM):
                        w0 = NMM * ch
                        for c in range(C):
                            nc.tensor.matmul(
                                counts_ps[32 * c:32 * c + 1, :],
                                ones[:, 0:1],
                                ind[:, c, w0:w0 + NMM],
                                start=(t == 0 and ch == 0),
                                stop=(t == HT - 1 and ch == W // NMM - 1),
                                tile_position=(0, 32 * c),
                            )

                out_sb = out_pool.tile([128, OUTW], F32)
                nc.scalar.copy(out_sb[:, 0:GW], gram_ps[:])
                nc.scalar.copy(out_sb[:, GW:OUTW], counts_ps[:])
                nc.gpsimd.dma_start(out=out[b], in_=out_sb[:])
    return nc


_NC = None


def _get_nc():
    global _NC
    if _NC is None:
        _NC = _build_nc()
    return _NC


def _finalize(stats: np.ndarray) -> np.float32:
    """stats: [B, 128, OUTW] f32 -> scalar loss.

    cols 0:160 = gram rows (c,wa) x cols (v,wb); cols 160:672 = counts
    partials on partitions {0,32,64,96}."""
    s = stats.astype(np.float64)
    gram = s[:, :, 0:GW].reshape(B, C, WB, NV, WB)
    diag = np.einsum('bcwvw->bcv', gram)
    S = diag[:, :, 0:E]                           # [B, 4, 4]
    T = diag[:, :, E]                             # [B, 4]
    counts = s[:, [32 * c for c in range(C)], GW:OUTW].sum(-1)   # [B, 4]
    with np.errstate(divide='ignore', invalid='ignore'):
        mu = S / counts[..., None]
        ssd = np.maximum(T - counts * (mu * mu).sum(-1), 0.0)
        nrm = np.sqrt(ssd)
        var = np.where(nrm > DELTA_V, (nrm - DELTA_V) ** 2, 0.0)
        L_var = var.mean()
        diff = mu[:, :, None, :] - mu[:, None, :, :]
        d2 = (diff * diff).sum(-1)
        eye = np.eye(C, dtype=bool)
        dist = np.sqrt(np.where(eye, 1.0, d2))
        dloss = np.where(eye, 0.0,
                         np.maximum(DELTA_D - dist, 0.0) ** 2).sum((-1, -2))
        L_dist = dloss.mean()
    return np.float32(L_var + L_dist)


def kernel(pred: np.ndarray, binary_label: np.ndarray,
           instance_label: np.ndarray) -> np.ndarray:
    from concourse.bass_utils import run_bass_kernel_spmd

    nc = _get_nc()
    in_maps = []
    for core in range(NCORES):
        b0 = core * B_LOC
        in_maps.append({
            "pred": np.ascontiguousarray(pred[b0:b0 + B_LOC], dtype=np.float32),
            "binary": np.ascontiguousarray(
                binary_label[b0:b0 + B_LOC], dtype=np.float32),
            "inst": np.ascontiguousarray(
                instance_label[b0:b0 + B_LOC], dtype=np.int32),
        })
    res = run_bass_kernel_spmd(nc, in_maps, core_ids=list(range(NCORES)))
    stats = np.concatenate([res.results[c]["out"] for c in range(NCORES)],
                           axis=0)              # [B, 128, OUTW]
    return _finalize(stats)


# revision 12
# speedup vs baseline: 1.1121x; 1.1121x over previous
"""Trainium2 Bass kernel for nn_Clustering (discriminative/lane clustering loss).

Strategy (8 NeuronCores, data parallel over batch, 2 images per core):
  Per image b the loss needs only 24 per-cluster statistics (c = 1..4):
    counts_c = sum_px [inst==c]                      (4)
    S_ce     = sum_px [inst==c] * binary * pred_e    (16)
    T_c      = sum_px [inst==c] * binary * |pred|^2  (4)

  Gram formulation: the 20 masked products S/T are inner products between
  mask planes q_c = [inst==c]*binary and value planes {pred_e, r=|pred|^2}
  over all pixels.  Feed the masks as the PE *stationary* (4 masks x 32
  w-offsets = 128 columns, reloaded per 32-column block) and stream the
  values as *moving* data [5 channels x 32 offsets = 160 columns]; the
  (wa==wb) diagonal of the accumulated [128,160] PSUM Gram holds the
  statistics.  The 16 mask*pred multiplies happen inside the systolic
  array, cutting DVE work ~3x vs elementwise product planes.

  Engine split per [128, 1024] tile:
    DVE : int->bf16 cast, v = inst*binary, 8 indicator compares (4 masked
          q_c on v, 4 raw ind_c on inst), 2 adds for r = sum_e pred_e^2
    ACT : binary + pred f32->bf16 casts, pred^2 squares, PSUM evacuation
    PE  : 32 Gram matmuls [128,(4,32)]^T @ [128,(5,32)] per tile, plus
          ones-column count reductions in 4 concurrent column groups
    DMA : 3 HWDGE loads per tile, 1 store per image
  The host reduces the Gram diagonal and evaluates the tiny [B,C,E] tail
  (means, variance hinge, pairwise center repulsion).
"""
import sys

sys.path.insert(0, '/opt/trn_rl_repo')

import numpy as np
from contextlib import ExitStack

import concourse.bass as bass
import concourse.mybir as mybir
import concourse.tile as tile
from concourse.alu_op_type import AluOpType
from concourse.vector_clock import ScopedClock

F32 = mybir.dt.float32
I32 = mybir.dt.int32
BF16 = mybir.dt.bfloat16

B, E, H, W = 16, 4, 512, 1024
NCORES = 8
B_LOC = B // NCORES          # images per core
C = 4                        # clusters 1..4 (background dropped)
HT = H // 128                # h-tiles per image
WB = 32                      # gram block width (4 masks x 32 = 128 stationary)
NB = W // WB                 # gram blocks per tile row
NV = 5                       # moving channels: pred_e x4, r
NMM = 512                    # counts matmul moving width (one PSUM bank)
GW = NV * WB                 # gram psum cols = 160
OUTW = GW + NMM              # out cols: gram 160 + counts 512

DELTA_V = 0.5
DELTA_D = 3.0

# ---------------------------------------------------------------------------
# Toolchain workaround: this walrus build rejects instructions carrying more
# than one sem-wait ("Too many sync wait commands").  Keep 1 wait per
# instruction and spill the rest onto preceding same-engine NOPs (the engine
# executes them in order, so semantics are unchanged).
_MAX_WAITS = 1


def _split_waits_prepend(tc, inst):
    si = getattr(inst, 'sync_info', None)
    if si is None or not si.on_wait or len(si.on_wait) <= _MAX_WAITS:
        return
    if inst.engine == mybir.EngineType.Unassigned:
        return
    waits = list(si.on_wait)
    si.on_wait = waits[:_MAX_WAITS]
    inst.sync_info = si
    for i in range(_MAX_WAITS, len(waits), _MAX_WAITS):
        nop = mybir.InstNoOp(name=tc.nc.get_next_instruction_name(),
                             text_hint="wait_split")
        nop.engine = inst.engine
        nop.sync_info = mybir.SyncInfo(on_wait=waits[i:i + _MAX_WAITS],
                                       on_update=[])
        tc._add_instruction(nop)


_orig_commit_and_lower = tile.TileContext._commit_and_lower


def _patched_commit_and_lower(self, inst, original_block, old_bb_map,
                              bb_to_exit_bb):
    _split_waits_prepend(self, inst)
    return _orig_commit_and_lower(self, inst, original_block, old_bb_map,
                                  bb_to_exit_bb)


tile.TileContext._commit_and_lower = _patched_commit_and_lower


def _patched_drain_and_barrier(self, tick_clock, wait_clock):
    nc = self.nc
    drain_inst = nc.sync.drain()
    wait_clock.add_sem_waits(
        drain_inst.ins, ScopedClock({None: tick_clock.global_clock})
    )
    si = drain_inst.ins.sync_info
    if si is not None and si.on_wait and len(si.on_wait) > _MAX_WAITS:
        waits = list(si.on_wait)
        si.on_wait = waits[:_MAX_WAITS]
        drain_inst.ins.sync_info = si
        extra = waits[_MAX_WAITS:]
        for i in range(0, len(extra), _MAX_WAITS):
            nop = nc.sync.nop()
            nop.ins.sync_info = mybir.SyncInfo(
                on_wait=extra[i:i + _MAX_WAITS], on_update=[]
            )
    nc.all_engine_barrier()
    assert self.sems is not None
    popped = nc._tile_sem_poison_stack.pop()
    assert popped is self._sem_poison
    nc.clear_and_free_semaphores(list(self.sems.allocated().values()))
    nc.all_engine_barrier()


tile.TileContext._drain_and_barrier = _patched_drain_and_barrier
# ---------------------------------------------------------------------------


def _build_nc():
    nc = bass.Bass()
    pred = nc.dram_tensor("pred", [B_LOC, E, H, W], F32, kind="ExternalInput")
    binary = nc.dram_tensor("binary", [B_LOC, H, W], F32, kind="ExternalInput")
    inst = nc.dram_tensor("inst", [B_LOC, H, W], I32, kind="ExternalInput")
    out = nc.dram_tensor("out", [B_LOC, 128, OUTW], F32, kind="ExternalOutput")

    with tile.TileContext(nc) as tc:
        with ExitStack() as ctx:
            const_pool = ctx.enter_context(tc.tile_pool(name="const", bufs=1))
            pred_pool = ctx.enter_context(tc.tile_pool(name="pred", bufs=3))
            in_pool = ctx.enter_context(tc.tile_pool(name="inp", bufs=3))
            bf_pool = ctx.enter_context(tc.tile_pool(name="bf", bufs=2))
            vals_pool = ctx.enter_context(tc.tile_pool(name="vals", bufs=2))
            mask_pool = ctx.enter_context(tc.tile_pool(name="mask", bufs=2))
            sq_pool = ctx.enter_context(tc.tile_pool(name="sq", bufs=2))
            ps_pool = ctx.enter_context(
                tc.tile_pool(name="ps", bufs=2, space="PSUM"))
            out_pool = ctx.enter_context(tc.tile_pool(name="outp", bufs=2))

            ones = const_pool.tile([128, 1], BF16)
            nc.vector.memset(ones[:], 1.0)

            for b in range(B_LOC):
                gram_ps = ps_pool.tile([128, GW], F32, tag="gram")
                counts_ps = ps_pool.tile([128, NMM], F32, tag="cnt")
                for t in range(HT):
                    h0 = 128 * t
                    inst_t = in_pool.tile([128, W], I32, tag="inst")
                    nc.sync.dma_start(
                        out=inst_t[:], in_=inst[b, h0:h0 + 128, :])
                    bin_t = in_pool.tile([128, W], F32, tag="bin")
                    nc.sync.dma_start(
                        out=bin_t[:], in_=binary[b, h0:h0 + 128, :])
                    pred_t = pred_pool.tile([128, E, W], F32, tag="pred")
                    nc.sync.dma_start(
                        out=pred_t[:],
                        in_=pred[b, :, h0:h0 + 128, :].rearrange(
                            "e h w -> h e w"),
                    )

                    # ACT: pred cast + squares; DVE: bin/inst casts, masks, r
                    vals = vals_pool.tile([128, NV, W], BF16, tag="vals")
                    nc.scalar.copy(vals[:, 0:E], pred_t[:])
                    sq = sq_pool.tile([128, E, W], BF16, tag="sq")
                    nc.scalar.activation(
                        sq[:], vals[:, 0:E],
                        mybir.ActivationFunctionType.Square)
                    bin_bf = bf_pool.tile([128, W], BF16, tag="binbf")
                    nc.vector.tensor_copy(bin_bf[:], bin_t[:])
                    inst_bf = bf_pool.tile([128, W], BF16, tag="instbf")
                    nc.vector.tensor_copy(inst_bf[:], inst_t[:])
                    v = bf_pool.tile([128, W], BF16, tag="v")
                    nc.vector.tensor_tensor(v[:], inst_bf[:], bin_bf[:],
                                            AluOpType.mult)
                    masks_q = mask_pool.tile([128, C, W], BF16, tag="mq")
                    for c in range(C):
                        nc.vector.tensor_scalar(
                            masks_q[:, c], v[:], float(c + 1), None,
                            AluOpType.is_equal)
                    ind = mask_pool.tile([128, C, W], BF16, tag="ind")
                    for c in range(C):
                        nc.vector.tensor_scalar(
                            ind[:, c], inst_bf[:], float(c + 1), None,
                            AluOpType.is_equal)
                    r2 = sq_pool.tile([128, 2, W], BF16, tag="r2")
                    nc.vector.tensor_tensor(r2[:], sq[:, 0:2], sq[:, 2:4],
                                            AluOpType.add)
                    nc.vector.tensor_tensor(vals[:, E], r2[:, 0], r2[:, 1],
                                            AluOpType.add)

                    # PE: Gram blocks — masks^T @ vals, diagonal-extracted on
                    # the host.  Each mask goes to its own 32-wide PE column
                    # group so the 4 matmuls of a block stream concurrently
                    # through separate XBUSes and the [128,32] stationary
                    # loads shrink 4x.  Accumulates over h-tiles and w-blocks.
                    for wb in range(NB):
                        w0 = WB * wb
                        for c in range(C):
                            nc.tensor.matmul(
                                gram_ps[32 * c:32 * c + WB, :],
                                masks_q[:, c, w0:w0 + WB],
                                vals[:, :, w0:w0 + WB],
                                start=(t == 0 and wb == 0),
                                stop=(t == HT - 1 and wb == NB - 1),
                                tile_position=(0, 32 * c),
                            )
                    # PE: counts — ones-column partition reductions of the
                    # 4 raw indicator planes, 4 concurrent column groups.
                    for ch in range(W // NMM):
                        w0 = NMM * ch
                        for c in range(C):
                            nc.tensor.matmul(
                                counts_ps[32 * c:32 * c + 1, :],
                                ones[:, 0:1],
                                ind[:, c, w0:w0 + NMM],
                                start=(t == 0 and ch == 0),
                                stop=(t == HT - 1 and ch == W // NMM - 1),
                                tile_position=(0, 32 * c),
                            )

                out_sb = out_pool.tile([128, OUTW], F32)
                nc.scalar.copy(out_sb[:, 0:GW], gram_ps[:])
                nc.scalar.copy(out_sb[:, GW:OUTW], counts_ps[:])
                nc.gpsimd.dma_start(out=out[b], in_=out_sb[:])
    return nc


_NC = None


def _get_nc():
    global _NC
    if _NC is None:
        _NC = _build_nc()
    return _NC


def _finalize(stats: np.ndarray) -> np.float32:
    """stats: [B, 128, OUTW] f32 -> scalar loss.

    cols 0:160 = gram rows (c,wa) x cols (v,wb); cols 160:672 = counts
    partials on partitions {0,32,64,96}."""
    s = stats.astype(np.float64)
    gram = s[:, :, 0:GW].reshape(B, C, WB, NV, WB)
    diag = np.einsum('bcwvw->bcv', gram)
    S = diag[:, :, 0:E]                           # [B, 4, 4]
    T = diag[:, :, E]                             # [B, 4]
    counts = s[:, [32 * c for c in range(C)], GW:OUTW].sum(-1)   # [B, 4]
    with np.errstate(divide='ignore', invalid='ignore'):
        mu = S / counts[..., None]
        ssd = np.maximum(T - counts * (mu * mu).sum(-1), 0.0)
        nrm = np.sqrt(ssd)
        var = np.where(nrm > DELTA_V, (nrm - DELTA_V) ** 2, 0.0)
        L_var = var.mean()
        diff = mu[:, :, None, :] - mu[:, None, :, :]
        d2 = (diff * diff).sum(-1)
        eye = np.eye(C, dtype=bool)
        dist = np.sqrt(np.where(eye, 1.0, d2))
        dloss = np.where(eye, 0.0,
                         np.maximum(DELTA_D - dist, 0.0) ** 2).sum((-1, -2))
        L_dist = dloss.mean()
    return np.float32(L_var + L_dist)


def kernel(pred: np.ndarray, binary_label: np.ndarray,
           instance_label: np.ndarray) -> np.ndarray:
    from concourse.bass_utils import run_bass_kernel_spmd

    nc = _get_nc()
    in_maps = []
    for core in range(NCORES):
        b0 = core * B_LOC
        in_maps.append({
            "pred": np.ascontiguousarray(pred[b0:b0 + B_LOC], dtype=np.float32),
            "binary": np.ascontiguousarray(
                binary_label[b0:b0 + B_LOC], dtype=np.float32),
            "inst": np.ascontiguousarray(
                instance_label[b0:b0 + B_LOC], dtype=np.int32),
        })
    res = run_bass_kernel_spmd(nc, in_maps, core_ids=list(range(NCORES)))
    stats = np.concatenate([res.results[c]["out"] for c in range(NCORES)],
                           axis=0)              # [B, 128, OUTW]
    return _finalize(stats)
